# revision 47
# baseline (speedup 1.0000x reference)
"""GPT-2 style attention block (B=8, S=1024, NX=1024, H=16, D=64) on 8 TRN2
NeuronCores, data-parallel over batch (one batch element per core).

Per-core math (batch element b):
  qkv = x @ w_attn + b_attn ; split q,k,v ; per head: softmax(causal(q k^T / 8)) v
  out = merge_heads @ w_proj + b_proj

v2: single merged software pipeline. The v1 kernel ran distinct phases
(QKV projection, V, attention, output projection); the attention phase was
bottlenecked by the Activation engine's exp throughput while the PE sat
half-idle (and dropped to its mid p-state). Here the per-head-pair work
(scores -> exp -> mask -> PV -> normalize) is interleaved with the
projection matmuls so every engine runs concurrently:

  PE:   transposes, QKV/V/proj matmuls, scores, PV     (~440k col-cycles)
  ACT:  exp over the causal area, qk PSUM->SBUF bias casts, xT copies
  DVE:  x/V/output copies, softmax denominator extract + reciprocal, normalize
  Pool: causal masks (affine_select), weight casting-DMA issue
  SP:   x / out / denominator-repartition DMA issue

Weights load via gpsimd-initiated casting DMAs (fp32 DRAM -> bf16 SBUF, no
staging pass).  The softmax denominator rides as a 65th V column through the
PV matmul; its [1,512] row is repartitioned to [128,4] by two tiny DMAs so
the DVE reciprocal (cost ~ columns, not elements) is cheap, then broadcast
across partitions with a K=1 ones matmul.  PSUM budget is exactly 8 banks:
2 (projection accum) + 4 (scores, [P,1024] double-buffered) + 2 (PV).

Layouts (all bf16 matmul operands, fp32 PSUM accumulation):
  xT  [128, kt*S + s]        x transposed, contraction (NX) on partitions
  qkT [128, nt*S + s]        (x@Wqk + b)^T; q blocks nt=0..7, k blocks 8..15;
                             head h lives at partitions (h%2)*64 .. +64 of
                             block (h//2) so each head slice is PE-ready
  v   [128, st, h, 65]       natural rows (sk on partitions) + ones column:
                             PV's last output row is the softmax denominator
  ET  [128, kt*S + sq]       exp(scores^T) per head, causal ranges only
  aT  [128, (h//2)*S + sq]   normalized attention output, lhsT for w_proj
"""

import numpy as np

B, S, NX, H = 8, 1024, 1024, 16
D = NX // H          # 64
P = 128              # partitions
ST = S // P          # 8 s-tiles
KT = NX // P         # 8 k-tiles
NQK = 2 * NX // P    # 16 n-tiles covering q and k
CH = 512             # matmul free-dim chunk (one PSUM bank of fp32)
NCH = S // CH        # 2 chunks
E = D + 1            # v columns per head incl. ones column
EV = H * E           # v row width per st block
NPAIR = H // 2       # 8 head pairs


def _split_excess_waits(nc):
    """Post-scheduling pass: the TPB instruction encodings carry at most one
    embedded sync-wait (and matmuls with their fused weight-load carry none),
    but Tile may attach several.  Move excess waits onto InstNoOp instructions
    inserted immediately before, on the same engine — semantically identical,
    but walrus can encode it."""
    import concourse.mybir as mybir

    SKIP = {
        "InstEventSemaphore",
        "InstUnconditionalBranch",
        "InstConditionalBranch",
        "InstRegisterMove",
        "InstRegisterAluOp",
    }
    # Types whose TPB encodings carry no embedded wait slot at all (fused
    # ldweights+matmul, gpsimd ucode ops) — move every wait to a NoOp.
    CAP0 = {
        "InstMatmult",
        "InstLdweights",
        "InstPartitionBroadcast",
        "InstAffineSelect",
        "InstTensorScalarAffineSelect",
        "InstCustomDveAnt",
        "InstISA",
    }
    n = 0
    for fn in nc.m.functions:
        for bb in fn.blocks:
            insts = bb.instructions
            inserts = []  # (index, [nops])
            for i, inst in enumerate(insts):
                tname = type(inst).__name__
                if tname in SKIP:
                    continue
                si = inst.sync_info
                if si is None or not si.on_wait:
                    continue
                waits = list(si.on_wait)
                cap = 0 if tname in CAP0 else 1
                if len(waits) <= cap:
                    continue
                keep, move = waits[:cap], waits[cap:]
                nops = []
                for w in move:
                    n += 1
                    nops.append(
                        mybir.InstNoOp(
                            name=f"wsplit-{n}",
                            text_hint="wsplit",
                            bass_nofuse=True,
                            engine=inst.engine,
                            sync_info=mybir.SyncInfo(on_wait=[w], on_update=[]),
                        )
                    )
                inst.sync_info = mybir.SyncInfo(
                    on_wait=keep,
                    on_update=list(si.on_update) if si.on_update else [],
                )
                inserts.append((i, nops))
            for i, nops in reversed(inserts):
                for nop in reversed(nops):
                    insts.insert(i, nop)
                    try:
                        nc.register_instruction(nop, overwrite=True)
                    except Exception:
                        pass
    return n


def build_nc(dbg=False):
    import concourse.bass as bass
    import concourse.mybir as mybir
    from concourse.tile import TileContext
    from concourse.masks import make_identity

    f32 = mybir.dt.float32
    bf16 = mybir.dt.bfloat16
    fp8 = mybir.dt.float8e4
    DR = mybir.MatmulPerfMode.DoubleRow
    Exp = mybir.ActivationFunctionType.Exp
    Copy = mybir.ActivationFunctionType.Copy
    Mult = mybir.AluOpType.mult

    nc = bass.Bass(target_bir_lowering=False)
    x_ext = nc.declare_dram_parameter("x", [S, NX], f32, isOutput=False)
    wa_ext = nc.declare_dram_parameter("w_attn", [NX, 3 * NX], f32, isOutput=False)
    ba_ext = nc.declare_dram_parameter("b_attn", [3 * NX], f32, isOutput=False)
    wp_ext = nc.declare_dram_parameter("w_proj", [NX, NX], f32, isOutput=False)
    bp_ext = nc.declare_dram_parameter("b_proj", [NX], f32, isOutput=False)
    out_ext = nc.declare_dram_parameter("out", [S, NX], f32, isOutput=True)
    if dbg:
        dbg_qkT = nc.declare_dram_parameter("dbg_qkT", [P, NQK * S], bf16, isOutput=True)
        dbg_v = nc.declare_dram_parameter("dbg_v", [P, ST * EV], bf16, isOutput=True)
        dbg_ET = nc.declare_dram_parameter("dbg_ET", [2, P, KT * S], bf16, isOutput=True)
        dbg_aT = nc.declare_dram_parameter("dbg_aT", [P, NPAIR * S], bf16, isOutput=True)

    wa_r = wa_ext.rearrange("(kt p) n -> p kt n", p=P)
    wp_r = wp_ext.rearrange("(kt p) n -> p kt n", p=P)

    with TileContext(nc) as tc:
        with (
            tc.tile_pool(name="const", bufs=1) as const,
            tc.tile_pool(name="small", bufs=3) as small,
            tc.tile_pool(name="p_xT", bufs=1) as p_xT,
            tc.tile_pool(name="p_qk", bufs=4) as p_qk,
            tc.tile_pool(name="p_v", bufs=1) as p_v,
            tc.tile_pool(name="p_aT", bufs=1) as p_aT,
            tc.tile_pool(name="p_wq", bufs=4) as p_wq,
            tc.tile_pool(name="p_wv", bufs=1) as p_wv,
        ):
            # ---------------- constants ----------------
            ident = const.tile([P, P], bf16)
            make_identity(nc, ident)
            ones_row = const.tile([1, P], bf16)
            nc.vector.memset(ones_row, 1.0)
            ba_v = const.tile([1, NX], bf16)    # b_attn[2048:3072] (v bias)
            nc.gpsimd.dma_start(out=ba_v, in_=ba_ext[2 * NX : 3 * NX].unsqueeze(0))
            ba_col = const.tile([P, NQK], f32)  # b_attn[:2048] column-major
            nc.sync.dma_start(
                out=ba_col, in_=ba_ext[0 : 2 * NX].rearrange("(nt p) -> p nt", p=P)
            )
            bp_row = const.tile([1, NX], bf16)
            nc.gpsimd.dma_start(out=bp_row, in_=bp_ext[:].unsqueeze(0))

            xT = p_xT.tile([P, KT * S], bf16)
            v_sb = p_v.tile([P, ST * EV], bf16)
            v_r = v_sb.rearrange("p (st h e) -> p st h e", h=H, e=E)
            aT = p_aT.tile([P, NPAIR * S], bf16)
            # ones column: PV's last output row is the softmax denominator
            nc.vector.memset(v_r[:, :, :, D : D + 1], 1.0)

            # w_attn nt-block q/k order interleaved with the pair schedule:
            # pair t needs q block nt=t and k block nt=8+t.
            def nt_for_pair(t):
                return (t, NPAIR + t)

            wq_tiles = {}

            def emit_wqk(t):
                # DMA with inline fp32->bf16 conversion (gpsimd-initiated)
                for nt in nt_for_pair(t):
                    wq = p_wq.tile([P, KT * P], bf16, name="wq")
                    nc.gpsimd.dma_start(
                        out=wq.rearrange("p (kt n) -> p kt n", n=P),
                        in_=wa_r[:, :, nt * P : (nt + 1) * P],
                    )
                    wq_tiles[nt] = wq

            # v weights: 8 kt slices, each [P, NX] fp32 -> bf16
            wv = p_wv.tile([P, KT * NX], bf16, name="wbig")

            def emit_wv(kt):
                nc.gpsimd.dma_start(
                    out=wv[:, kt * NX : (kt + 1) * NX],
                    in_=wa_r[:, kt, 2 * NX : 3 * NX],
                )

            # ---------------- phase A: x -> xT ----------------
            with (
                tc.tile_pool(name="p_ET", bufs=4) as p_ET,
                tc.tile_pool(name="ps_b", bufs=1, space="PSUM") as ps_b,
                tc.tile_pool(name="ps_s", bufs=1, space="PSUM") as ps_s,
                tc.tile_pool(name="ps_u", bufs=1, space="PSUM") as ps_u,
            ):
                qk_tiles = {}

                def emit_b_block(nt, dst, base, cs=tuple(range(NCH))):
                    """dst[:, base:base+S] = (x @ w_attn[:, nt])^T + b."""
                    wq = wq_tiles[nt]
                    for c in cs:
                        pm = ps_b.tile([P, CH], f32, name="pm_b", bufs=2)
                        for kt in range(KT):
                            nc.tensor.matmul(
                                out=pm,
                                lhsT=wq[:, kt * P : (kt + 1) * P],
                                rhs=xT[:, kt * S + c * CH : kt * S + (c + 1) * CH],
                                start=(kt == 0),
                                stop=(kt == KT - 1),
                            )
                        nc.scalar.add(
                            out=dst[:, base + c * CH : base + (c + 1) * CH],
                            in_=pm,
                            add=ba_col[:, nt : nt + 1],
                        )

                xbf = p_xT.tile([P, ST * NX], bf16)
                # pair-0 q/k weights first on the gpsimd ring, then x via
                # casting DMAs (fp32 DRAM -> bf16 SBUF, no staging pass)
                emit_wqk(0)
                for st in range(ST):
                    nc.gpsimd.dma_start(
                        out=xbf[:, st * NX : (st + 1) * NX],
                        in_=x_ext[st * P : (st + 1) * P, :],
                    )
                emit_wqk(1)
                for kt in range(KT):
                    emit_wv(kt)

                def emit_transposes(sts):
                    for st in sts:
                        pt = ps_b.tile([P, KT * P], bf16, name="pm_b", bufs=2)
                        for kt in range(KT):
                            nc.tensor.transpose(
                                out=pt[:, kt * P : (kt + 1) * P],
                                in_=xbf[:, st * NX + kt * P : st * NX + (kt + 1) * P],
                                identity=ident,
                            )
                        nc.scalar.copy(
                            out=bass.AP(
                                tensor=xT.tensor,
                                offset=xT.offset + st * P,
                                ap=[[KT * S, P], [S, KT], [1, P]],
                            ),
                            in_=pt.rearrange("p (kt n) -> p kt n", n=P),
                        )

                emit_transposes(range(0, 4))
                # pair 0's q projection on tokens 0-511 can start as soon as
                # the first four transposes land
                qk_tiles[0] = p_qk.tile([P, 2 * S], bf16, name="qk")
                emit_b_block(nt_for_pair(0)[0], qk_tiles[0], 0, cs=(0,))
                emit_transposes(range(4, ST))

                wp_sb = p_wv.tile([P, KT * NX], bf16, name="wbig")

                def emit_wp(kt):
                    nc.gpsimd.dma_start(
                        out=wp_sb[:, kt * NX : (kt + 1) * NX], in_=wp_r[:, kt, :]
                    )

                def emit_v_block(st):
                    """v rows for sk-block st, bias added, ones col preset."""
                    for c in range(NCH):
                        pm = ps_b.tile([P, CH], f32, name="pm_b", bufs=2)
                        for kt in range(KT):
                            nc.tensor.matmul(
                                out=pm,
                                lhsT=xT[:, kt * S + st * P : kt * S + (st + 1) * P],
                                rhs=wv[:, kt * NX + c * CH : kt * NX + (c + 1) * CH],
                                start=(kt == 0),
                                stop=False,
                            )
                        nc.tensor.matmul(
                            out=pm,
                            lhsT=ones_row,
                            rhs=ba_v[:, c * CH : (c + 1) * CH],
                            start=False,
                            stop=True,
                        )
                        nc.vector.tensor_copy(
                            out=v_r[:, st, 8 * c : 8 * (c + 1), 0:D],
                            in_=pm.rearrange("p (h d) -> p h d", d=D),
                        )

                ET = {}

                def emit_scores_chunk(h, kts):
                    """scoresT + exp for k-blocks kts of head h into ET[h]."""
                    po = (h % 2) * 64
                    qk = qk_tiles[h // 2]
                    for kt in kts:
                        lo = kt * P  # first causal sq column
                        pm = ps_s.tile([P, 2 * CH], f32, name="pm_s", bufs=2)
                        for c in range(lo // CH, NCH):
                            off = max(0, lo - c * CH)
                            nc.tensor.matmul(
                                out=pm[:, c * CH + off : (c + 1) * CH],
                                lhsT=qk[
                                    po : po + 64,
                                    S + kt * P : S + (kt + 1) * P,
                                ],
                                rhs=qk[
                                    po : po + 64,
                                    c * CH + off : (c + 1) * CH,
                                ],
                                start=True,
                                stop=True,
                            )
                        nc.scalar.activation(
                            out=ET[h][:, kt * S + lo : (kt + 1) * S],
                            in_=pm[:, lo:S],
                            func=Exp,
                            scale=0.125,
                        )

                def emit_mask(h, lo, hi):
                    """Zero the non-causal (sq < sk) part of diagonal blocks
                    kt in [lo, hi) of ET[h] in place (gpsimd affine iota:
                    keep where sq - sk >= 0)."""
                    diag = bass.AP(
                        tensor=ET[h].tensor,
                        offset=ET[h].offset + lo * (S + P),
                        ap=[[KT * S, P], [S + P, hi - lo], [1, P]],
                    )
                    nc.gpsimd.affine_select(
                        out=diag,
                        in_=diag,
                        compare_op=mybir.AluOpType.is_ge,
                        fill=0.0,
                        base=0,
                        pattern=[[0, hi - lo], [1, P]],
                        channel_multiplier=-1,
                    )

                def emit_pv_chunk(h, c):
                    """pu rows 0:64 = v_h^T E^T chunk; rows 64:128 = the
                    denominator replicated 64-wide (lhsT = [v_h | ones] via a
                    two-region AP) — same stream length, free broadcast."""
                    pu = ps_u.tile([E, CH], f32, name="pu", bufs=2)
                    kt_hi = ((c + 1) * CH) // P
                    for kt in range(kt_hi):
                        off = max(0, P * kt - c * CH)
                        nc.tensor.matmul(
                            out=pu[:, off:CH],
                            lhsT=v_r[:, kt, h, :],
                            rhs=ET[h][:, kt * S + c * CH + off : kt * S + (c + 1) * CH],
                            start=(kt == 0),
                            stop=(kt == kt_hi - 1),
                        )
                    return pu

                def emit_finish(h, c, pu):
                    """aT chunk = pu rows * 1/denominator row (pu[D]).

                    DVE reciprocal cost is driven by the COLUMN count (~8ns/
                    col regardless of partitions), so repartition the [1,CH]
                    denominator row to [128, CH/128] with two tiny DMAs, then
                    broadcast back across partitions with a K=1 ones matmul."""
                    t, po = h // 2, (h % 2) * 64
                    # copy numerators + denominator row out, freeing the pu
                    # PSUM bank immediately; the reciprocal chain then runs
                    # fully decoupled (aT isn't read until the projection).
                    num_sb = small.tile([64, CH], bf16, name="num_sb", bufs=4)
                    nc.vector.tensor_copy(out=num_sb, in_=pu[0:D, :])
                    r_sb = small.tile([1, CH], f32, name="r_sb", bufs=4)
                    nc.vector.tensor_copy(out=r_sb, in_=pu[D : D + 1, :])
                    r_wide = small.tile([P, CH // P], f32, name="r_wide", bufs=4)
                    nc.sync.dma_start(out=r_wide, in_=r_sb)
                    rw_rec = small.tile([P, CH // P], bf16, name="rw_rec", bufs=4)
                    with nc.allow_low_precision(
                        reason="softmax denominators; bf16 ok at 2e-2 gate"
                    ):
                        nc.vector.reciprocal(out=rw_rec, in_=r_wide)
                    r_row = small.tile([1, CH], bf16, name="r_row", bufs=4)
                    nc.sync.dma_start(out=r_row, in_=rw_rec)
                    pr = ps_s.tile([64, CH], f32, name="pm_s", bufs=2)
                    nc.tensor.matmul(
                        out=pr,
                        lhsT=ones_row[:, 0:64],
                        rhs=r_row,
                        start=True,
                        stop=True,
                    )
                    recipB = small.tile([64, CH], bf16, name="recipB", bufs=4)
                    nc.vector.tensor_copy(out=recipB, in_=pr)
                    nc.vector.tensor_mul(
                        out=aT[po : po + 64, t * S + c * CH : t * S + (c + 1) * CH],
                        in0=num_sb,
                        in1=recipB,
                    )

                # ---------------- the merged pipeline ----------------
                # Emission order is execution order per engine, and a tile
                # slot's new writer can only be ordered after reads that are
                # already emitted.  So: all v blocks go in iterations 0-1
                # (PV c1 needs every sk block), PV c0 chunks drain one pair
                # behind, PV c1 chunks two pairs behind — emitted BEFORE the
                # pair that reuses their ET slot (ET bufs=4) allocates.
                def drain(h, c):
                    emit_finish(h, c, emit_pv_chunk(h, c))

                for t in range(NPAIR):
                    # weight pipeline: fetch pair t+2's qk slices; w_proj
                    # spread over t=4..7
                    if t + 2 < NPAIR:
                        emit_wqk(t + 2)
                    if 4 <= t < 8:
                        emit_wp(2 * (t - 4))
                        emit_wp(2 * (t - 4) + 1)

                    # free the ET slots this pair will take over
                    if t >= 2:
                        drain(2 * (t - 2), 1)
                        drain(2 * (t - 2) + 1, 1)

                    h0, h1 = 2 * t, 2 * t + 1
                    ET[h0] = p_ET.tile([P, KT * S], bf16, name="ET")
                    ET[h1] = p_ET.tile([P, KT * S], bf16, name="ET")
                    if t > 0:
                        qk_tiles[t] = p_qk.tile([P, 2 * S], bf16, name="qk")

                    qnt, knt = nt_for_pair(t)
                    emit_b_block(qnt, qk_tiles[t], 0, cs=(1,) if t == 0 else (0, 1))
                    emit_b_block(knt, qk_tiles[t], S)
                    if t < 2:
                        for st in range(4 * t, 4 * t + 4):
                            emit_v_block(st)

                    # previous pair's first-half PV fills the PE while this
                    # pair's qk bias-casts land; breadth-first over the two
                    # heads so each head's kt4-7 scores trail their own exp
                    # chunks by a full head's worth of work
                    if t >= 1:
                        drain(2 * (t - 1), 0)
                    emit_scores_chunk(h0, range(0, 4))
                    emit_mask(h0, 0, 4)
                    emit_scores_chunk(h1, range(0, 4))
                    emit_mask(h1, 0, 4)
                    if t >= 1:
                        drain(2 * (t - 1) + 1, 0)
                    emit_scores_chunk(h0, range(4, KT))
                    emit_mask(h0, 4, KT)
                    emit_scores_chunk(h1, range(4, KT))
                    emit_mask(h1, 4, KT)

                def emit_d(st):
                    for cd in range(NCH):
                        pm = ps_b.tile([P, CH], f32, name="pm_b", bufs=2)
                        for kt in range(KT):
                            nc.tensor.matmul(
                                out=pm,
                                lhsT=aT[:, kt * S + st * P : kt * S + (st + 1) * P],
                                rhs=wp_sb[:, kt * NX + cd * CH : kt * NX + (cd + 1) * CH],
                                start=(kt == 0),
                                stop=False,
                            )
                        nc.tensor.matmul(
                            out=pm,
                            lhsT=ones_row,
                            rhs=bp_row[:, cd * CH : (cd + 1) * CH],
                            start=False,
                            stop=True,
                        )
                        stage = small.tile([P, CH], f32, name="stage", bufs=2)
                        nc.vector.tensor_copy(out=stage, in_=pm)
                        nc.sync.dma_start(
                            out=out_ext[st * P : (st + 1) * P, cd * CH : (cd + 1) * CH],
                            in_=stage,
                        )

                # tail: pair 7 c0 first (completes aT cols 0..511 for all
                # heads), then the projection's first half interleaved with
                # the remaining c1 drains.
                drain(H - 2, 0)
                drain(H - 1, 0)
                emit_d(0)
                drain(H - 4, 1)
                emit_d(1)
                drain(H - 3, 1)
                emit_d(2)
                drain(H - 2, 1)
                emit_d(3)
                drain(H - 1, 1)
                for st in range(4, ST):
                    emit_d(st)

                if dbg:
                    # only pairs 4-7's qk tiles are still live (bufs=4)
                    for t in range(4, NPAIR):
                        nc.sync.dma_start(
                            out=dbg_qkT[:, (t - 4) * 2 * S : (t - 3) * 2 * S],
                            in_=qk_tiles[t],
                        )
                    nc.sync.dma_start(out=dbg_v[:, :], in_=v_sb)
                    nc.sync.dma_start(out=dbg_ET[0, :, :], in_=ET[H - 2])
                    nc.sync.dma_start(out=dbg_ET[1, :, :], in_=ET[H - 1])
                    nc.sync.dma_start(out=dbg_aT[:, :], in_=aT)

    _split_excess_waits(nc)
    return nc


def _enable_ldw_opt():
    """walrus is invoked with --enable-ldw-opt=false on this path; turning it
    on lets codegen elide redundant LDWEIGHTS for back-to-back matmuls that
    share a stationary operand."""
    import concourse.bass_utils as bu

    if getattr(bu, "_ldw_opt_patched", False):
        return
    orig = bu.run_command

    def patched(cmd, **kw):
        cmd = [
            c.replace("--enable-ldw-opt=false", "--enable-ldw-opt=true")
            if isinstance(c, str)
            else c
            for c in cmd
        ]
        return orig(cmd, **kw)

    bu.run_command = patched
    bu._ldw_opt_patched = True


def run(inputs, trace=False, **kwargs):
    """Run the SPMD kernel on 8 cores; returns (output, BassKernelResults)."""
    from concourse.bass_utils import run_bass_kernel_spmd

    x = np.ascontiguousarray(np.asarray(inputs["x"], dtype=np.float32))
    w_attn = np.ascontiguousarray(np.asarray(inputs["w_attn"], dtype=np.float32))
    b_attn = np.ascontiguousarray(np.asarray(inputs["b_attn"], dtype=np.float32))
    w_proj = np.ascontiguousarray(np.asarray(inputs["w_proj"], dtype=np.float32))
    b_proj = np.ascontiguousarray(np.asarray(inputs["b_proj"], dtype=np.float32))

    nc = build_nc()
    in_maps = [
        {
            "x": x[b],
            "w_attn": w_attn,
            "b_attn": b_attn,
            "w_proj": w_proj,
            "b_proj": b_proj,
        }
        for b in range(B)
    ]
    res = run_bass_kernel_spmd(
        nc, in_maps, core_ids=list(range(B)), trace=trace, **kwargs
    )
    out = np.stack([res.results[i]["out"] for i in range(B)], axis=0)
    return out.astype(np.float32), res


def kernel(**inputs):
    out, _ = run(inputs)
    return out


# revision 48
# speedup vs baseline: 1.0202x; 1.0202x over previous
"""GPT-2 style attention block (B=8, S=1024, NX=1024, H=16, D=64) on 8 TRN2
NeuronCores, data-parallel over batch (one batch element per core).

Per-core math (batch element b):
  qkv = x @ w_attn + b_attn ; split q,k,v ; per head: softmax(causal(q k^T / 8)) v
  out = merge_heads @ w_proj + b_proj

v2: single merged software pipeline. The v1 kernel ran distinct phases
(QKV projection, V, attention, output projection); the attention phase was
bottlenecked by the Activation engine's exp throughput while the PE sat
half-idle (and dropped to its mid p-state). Here the per-head-pair work
(scores -> exp -> mask -> PV -> normalize) is interleaved with the
projection matmuls so every engine runs concurrently:

  PE:   transposes, QKV/V/proj matmuls, scores, PV     (~440k col-cycles)
  ACT:  exp over the causal area, qk PSUM->SBUF bias casts, xT copies
  DVE:  x/V/output copies, softmax denominator extract + reciprocal, normalize
  Pool: causal masks (affine_select), weight casting-DMA issue
  SP:   x / out / denominator-repartition DMA issue

Weights load via gpsimd-initiated casting DMAs (fp32 DRAM -> bf16 SBUF, no
staging pass).  The softmax denominator rides as a 65th V column through the
PV matmul; its [1,512] row is repartitioned to [128,4] by two tiny DMAs so
the DVE reciprocal (cost ~ columns, not elements) is cheap, then broadcast
across partitions with a K=1 ones matmul.  PSUM budget is exactly 8 banks:
2 (projection accum) + 4 (scores, [P,1024] double-buffered) + 2 (PV).

Layouts (all bf16 matmul operands, fp32 PSUM accumulation):
  xT  [128, kt*S + s]        x transposed, contraction (NX) on partitions
  qkT [128, nt*S + s]        (x@Wqk + b)^T; q blocks nt=0..7, k blocks 8..15;
                             head h lives at partitions (h%2)*64 .. +64 of
                             block (h//2) so each head slice is PE-ready
  v   [128, st, h, 65]       natural rows (sk on partitions) + ones column:
                             PV's last output row is the softmax denominator
  ET  [128, kt*S + sq]       exp(scores^T) per head, causal ranges only
  aT  [128, (h//2)*S + sq]   normalized attention output, lhsT for w_proj
"""

import numpy as np

B, S, NX, H = 8, 1024, 1024, 16
D = NX // H          # 64
P = 128              # partitions
ST = S // P          # 8 s-tiles
KT = NX // P         # 8 k-tiles
NQK = 2 * NX // P    # 16 n-tiles covering q and k
CH = 512             # matmul free-dim chunk (one PSUM bank of fp32)
NCH = S // CH        # 2 chunks
E = D + 1            # v columns per head incl. ones column
EV = H * E           # v row width per st block
NPAIR = H // 2       # 8 head pairs


def _split_excess_waits(nc):
    """Post-scheduling pass: the TPB instruction encodings carry at most one
    embedded sync-wait (and matmuls with their fused weight-load carry none),
    but Tile may attach several.  Move excess waits onto InstNoOp instructions
    inserted immediately before, on the same engine — semantically identical,
    but walrus can encode it."""
    import concourse.mybir as mybir

    SKIP = {
        "InstEventSemaphore",
        "InstUnconditionalBranch",
        "InstConditionalBranch",
        "InstRegisterMove",
        "InstRegisterAluOp",
    }
    # Types whose TPB encodings carry no embedded wait slot at all (fused
    # ldweights+matmul, gpsimd ucode ops) — move every wait to a NoOp.
    CAP0 = {
        "InstMatmult",
        "InstLdweights",
        "InstPartitionBroadcast",
        "InstAffineSelect",
        "InstTensorScalarAffineSelect",
        "InstCustomDveAnt",
        "InstISA",
    }
    n = 0
    for fn in nc.m.functions:
        for bb in fn.blocks:
            insts = bb.instructions
            inserts = []  # (index, [nops])
            for i, inst in enumerate(insts):
                tname = type(inst).__name__
                if tname in SKIP:
                    continue
                si = inst.sync_info
                if si is None or not si.on_wait:
                    continue
                waits = list(si.on_wait)
                cap = 0 if tname in CAP0 else 1
                if len(waits) <= cap:
                    continue
                keep, move = waits[:cap], waits[cap:]
                nops = []
                for w in move:
                    n += 1
                    nops.append(
                        mybir.InstNoOp(
                            name=f"wsplit-{n}",
                            text_hint="wsplit",
                            bass_nofuse=True,
                            engine=inst.engine,
                            sync_info=mybir.SyncInfo(on_wait=[w], on_update=[]),
                        )
                    )
                inst.sync_info = mybir.SyncInfo(
                    on_wait=keep,
                    on_update=list(si.on_update) if si.on_update else [],
                )
                inserts.append((i, nops))
            for i, nops in reversed(inserts):
                for nop in reversed(nops):
                    insts.insert(i, nop)
                    try:
                        nc.register_instruction(nop, overwrite=True)
                    except Exception:
                        pass
    return n


def build_nc(dbg=False):
    import concourse.bass as bass
    import concourse.mybir as mybir
    from concourse.tile import TileContext
    from concourse.masks import make_identity

    f32 = mybir.dt.float32
    bf16 = mybir.dt.bfloat16
    fp8 = mybir.dt.float8e4
    DR = mybir.MatmulPerfMode.DoubleRow
    Exp = mybir.ActivationFunctionType.Exp
    Copy = mybir.ActivationFunctionType.Copy
    Mult = mybir.AluOpType.mult

    nc = bass.Bass(target_bir_lowering=False)
    x_ext = nc.declare_dram_parameter("x", [S, NX], f32, isOutput=False)
    wa_ext = nc.declare_dram_parameter("w_attn", [NX, 3 * NX], f32, isOutput=False)
    ba_ext = nc.declare_dram_parameter("b_attn", [3 * NX], f32, isOutput=False)
    wp_ext = nc.declare_dram_parameter("w_proj", [NX, NX], f32, isOutput=False)
    bp_ext = nc.declare_dram_parameter("b_proj", [NX], f32, isOutput=False)
    out_ext = nc.declare_dram_parameter("out", [S, NX], f32, isOutput=True)
    if dbg:
        dbg_qkT = nc.declare_dram_parameter("dbg_qkT", [P, NQK * S], bf16, isOutput=True)
        dbg_v = nc.declare_dram_parameter("dbg_v", [P, ST * EV], bf16, isOutput=True)
        dbg_ET = nc.declare_dram_parameter("dbg_ET", [2, P, KT * S], bf16, isOutput=True)
        dbg_aT = nc.declare_dram_parameter("dbg_aT", [P, NPAIR * S], bf16, isOutput=True)

    wa_r = wa_ext.rearrange("(kt p) n -> p kt n", p=P)
    wp_r = wp_ext.rearrange("(kt p) n -> p kt n", p=P)

    with TileContext(nc) as tc:
        with (
            tc.tile_pool(name="const", bufs=1) as const,
            tc.tile_pool(name="small", bufs=3) as small,
            tc.tile_pool(name="p_xT", bufs=1) as p_xT,
            tc.tile_pool(name="p_qk", bufs=4) as p_qk,
            tc.tile_pool(name="p_v", bufs=1) as p_v,
            tc.tile_pool(name="p_aT", bufs=1) as p_aT,
            tc.tile_pool(name="p_wq", bufs=4) as p_wq,
            tc.tile_pool(name="p_wv", bufs=1) as p_wv,
        ):
            # ---------------- constants ----------------
            ident = const.tile([P, P], bf16)
            make_identity(nc, ident)
            ones_row = const.tile([1, P], bf16)
            nc.vector.memset(ones_row, 1.0)
            ba_v = const.tile([1, NX], bf16)    # b_attn[2048:3072] (v bias)
            nc.gpsimd.dma_start(out=ba_v, in_=ba_ext[2 * NX : 3 * NX].unsqueeze(0))
            ba_col = const.tile([P, NQK], f32)  # b_attn[:2048] column-major
            nc.sync.dma_start(
                out=ba_col, in_=ba_ext[0 : 2 * NX].rearrange("(nt p) -> p nt", p=P)
            )
            bp_row = const.tile([1, NX], bf16)
            nc.gpsimd.dma_start(out=bp_row, in_=bp_ext[:].unsqueeze(0))

            xT = p_xT.tile([P, KT * S], bf16)
            v_sb = p_v.tile([P, ST * EV], bf16)
            v_r = v_sb.rearrange("p (st h e) -> p st h e", h=H, e=E)
            aT = p_aT.tile([P, NPAIR * S], bf16)
            # ones column: PV's last output row is the softmax denominator
            nc.vector.memset(v_r[:, :, :, D : D + 1], 1.0)

            # w_attn nt-block q/k order interleaved with the pair schedule:
            # pair t needs q block nt=t and k block nt=8+t.
            def nt_for_pair(t):
                return (t, NPAIR + t)

            wq_tiles = {}

            def emit_wqk(t):
                # DMA with inline fp32->bf16 conversion (gpsimd-initiated)
                for nt in nt_for_pair(t):
                    wq = p_wq.tile([P, KT * P], bf16, name="wq")
                    nc.gpsimd.dma_start(
                        out=wq.rearrange("p (kt n) -> p kt n", n=P),
                        in_=wa_r[:, :, nt * P : (nt + 1) * P],
                    )
                    wq_tiles[nt] = wq

            # v weights: 8 kt slices, each [P, NX] fp32 -> bf16
            wv = p_wv.tile([P, KT * NX], bf16, name="wbig")

            def emit_wv(kt):
                nc.gpsimd.dma_start(
                    out=wv[:, kt * NX : (kt + 1) * NX],
                    in_=wa_r[:, kt, 2 * NX : 3 * NX],
                )

            # ---------------- phase A: x -> xT ----------------
            with (
                tc.tile_pool(name="p_ET", bufs=4) as p_ET,
                tc.tile_pool(name="ps_b", bufs=1, space="PSUM") as ps_b,
                tc.tile_pool(name="ps_s", bufs=1, space="PSUM") as ps_s,
                tc.tile_pool(name="ps_u", bufs=1, space="PSUM") as ps_u,
            ):
                qk_tiles = {}

                def emit_b_block(nt, dst, base, cs=tuple(range(NCH))):
                    """dst[:, base:base+S] = (x @ w_attn[:, nt])^T + b."""
                    wq = wq_tiles[nt]
                    for c in cs:
                        pm = ps_b.tile([P, CH], f32, name="pm_b", bufs=2)
                        for kt in range(KT):
                            nc.tensor.matmul(
                                out=pm,
                                lhsT=wq[:, kt * P : (kt + 1) * P],
                                rhs=xT[:, kt * S + c * CH : kt * S + (c + 1) * CH],
                                start=(kt == 0),
                                stop=(kt == KT - 1),
                            )
                        nc.scalar.add(
                            out=dst[:, base + c * CH : base + (c + 1) * CH],
                            in_=pm,
                            add=ba_col[:, nt : nt + 1],
                        )

                xbf = p_xT.tile([P, ST * NX], bf16)
                # x lands fp32 in the first two ET-pool slots (dead until
                # pair 0/1's exp; the rotation WAR orders that) and is
                # pre-cast to bf16 on the idle DVE for 1-cycle transposes
                x_sb = [
                    p_ET.tile([P, 4 * NX], f32, name="ET"),
                    p_ET.tile([P, 4 * NX], f32, name="ET"),
                ]
                for st in range(ST):
                    ring = nc.sync if st % 2 == 0 else nc.scalar
                    ring.dma_start(
                        out=x_sb[st // 4][:, (st % 4) * NX : (st % 4 + 1) * NX],
                        in_=x_ext[st * P : (st + 1) * P, :],
                    )
                emit_wqk(0)
                emit_wqk(1)
                for kt in range(KT):
                    emit_wv(kt)

                def emit_transposes(sts):
                    for st in sts:
                        nc.vector.tensor_copy(
                            out=xbf[:, st * NX : (st + 1) * NX],
                            in_=x_sb[st // 4][:, (st % 4) * NX : (st % 4 + 1) * NX],
                        )
                        pt = ps_b.tile([P, KT * P], bf16, name="pm_b", bufs=2)
                        for kt in range(KT):
                            nc.tensor.transpose(
                                out=pt[:, kt * P : (kt + 1) * P],
                                in_=xbf[:, st * NX + kt * P : st * NX + (kt + 1) * P],
                                identity=ident,
                            )
                        nc.scalar.copy(
                            out=bass.AP(
                                tensor=xT.tensor,
                                offset=xT.offset + st * P,
                                ap=[[KT * S, P], [S, KT], [1, P]],
                            ),
                            in_=pt.rearrange("p (kt n) -> p kt n", n=P),
                        )

                emit_transposes(range(0, 4))
                # pair 0's q projection on tokens 0-511 can start as soon as
                # the first four transposes land
                qk_tiles[0] = p_qk.tile([P, 2 * S], bf16, name="qk")
                emit_b_block(nt_for_pair(0)[0], qk_tiles[0], 0, cs=(0,))
                emit_transposes(range(4, ST))

                wp_sb = p_wv.tile([P, KT * NX], bf16, name="wbig")

                def emit_wp(kt):
                    nc.gpsimd.dma_start(
                        out=wp_sb[:, kt * NX : (kt + 1) * NX], in_=wp_r[:, kt, :]
                    )

                def emit_v_block(st):
                    """v rows for sk-block st, bias added, ones col preset."""
                    for c in range(NCH):
                        pm = ps_b.tile([P, CH], f32, name="pm_b", bufs=2)
                        for kt in range(KT):
                            nc.tensor.matmul(
                                out=pm,
                                lhsT=xT[:, kt * S + st * P : kt * S + (st + 1) * P],
                                rhs=wv[:, kt * NX + c * CH : kt * NX + (c + 1) * CH],
                                start=(kt == 0),
                                stop=False,
                            )
                        nc.tensor.matmul(
                            out=pm,
                            lhsT=ones_row,
                            rhs=ba_v[:, c * CH : (c + 1) * CH],
                            start=False,
                            stop=True,
                        )
                        nc.vector.tensor_copy(
                            out=v_r[:, st, 8 * c : 8 * (c + 1), 0:D],
                            in_=pm.rearrange("p (h d) -> p h d", d=D),
                        )

                ET = {}

                def emit_scores_chunk(h, kts):
                    """scoresT + exp for k-blocks kts of head h into ET[h]."""
                    po = (h % 2) * 64
                    qk = qk_tiles[h // 2]
                    for kt in kts:
                        lo = kt * P  # first causal sq column
                        pm = ps_s.tile([P, 2 * CH], f32, name="pm_s", bufs=2)
                        for c in range(lo // CH, NCH):
                            off = max(0, lo - c * CH)
                            nc.tensor.matmul(
                                out=pm[:, c * CH + off : (c + 1) * CH],
                                lhsT=qk[
                                    po : po + 64,
                                    S + kt * P : S + (kt + 1) * P,
                                ],
                                rhs=qk[
                                    po : po + 64,
                                    c * CH + off : (c + 1) * CH,
                                ],
                                start=True,
                                stop=True,
                            )
                        nc.scalar.activation(
                            out=ET[h][:, kt * S + lo : (kt + 1) * S],
                            in_=pm[:, lo:S],
                            func=Exp,
                            scale=0.125,
                        )

                def emit_mask(h, lo, hi):
                    """Zero the non-causal (sq < sk) part of diagonal blocks
                    kt in [lo, hi) of ET[h] in place (gpsimd affine iota:
                    keep where sq - sk >= 0)."""
                    diag = bass.AP(
                        tensor=ET[h].tensor,
                        offset=ET[h].offset + lo * (S + P),
                        ap=[[KT * S, P], [S + P, hi - lo], [1, P]],
                    )
                    nc.gpsimd.affine_select(
                        out=diag,
                        in_=diag,
                        compare_op=mybir.AluOpType.is_ge,
                        fill=0.0,
                        base=0,
                        pattern=[[0, hi - lo], [1, P]],
                        channel_multiplier=-1,
                    )

                def emit_pv_chunk(h, c):
                    """pu rows 0:64 = v_h^T E^T chunk; rows 64:128 = the
                    denominator replicated 64-wide (lhsT = [v_h | ones] via a
                    two-region AP) — same stream length, free broadcast."""
                    pu = ps_u.tile([E, CH], f32, name="pu", bufs=2)
                    kt_hi = ((c + 1) * CH) // P
                    for kt in range(kt_hi):
                        off = max(0, P * kt - c * CH)
                        nc.tensor.matmul(
                            out=pu[:, off:CH],
                            lhsT=v_r[:, kt, h, :],
                            rhs=ET[h][:, kt * S + c * CH + off : kt * S + (c + 1) * CH],
                            start=(kt == 0),
                            stop=(kt == kt_hi - 1),
                        )
                    return pu

                def emit_finish(h, c, pu):
                    """aT chunk = pu rows * 1/denominator row (pu[D]).

                    DVE reciprocal cost is driven by the COLUMN count (~8ns/
                    col regardless of partitions), so repartition the [1,CH]
                    denominator row to [128, CH/128] with two tiny DMAs, then
                    broadcast back across partitions with a K=1 ones matmul."""
                    t, po = h // 2, (h % 2) * 64
                    # copy numerators + denominator row out, freeing the pu
                    # PSUM bank immediately; the reciprocal chain then runs
                    # fully decoupled (aT isn't read until the projection).
                    num_sb = small.tile([64, CH], bf16, name="num_sb", bufs=4)
                    nc.vector.tensor_copy(out=num_sb, in_=pu[0:D, :])
                    r_sb = small.tile([1, CH], f32, name="r_sb", bufs=4)
                    nc.vector.tensor_copy(out=r_sb, in_=pu[D : D + 1, :])
                    r_wide = small.tile([P, CH // P], f32, name="r_wide", bufs=4)
                    nc.sync.dma_start(out=r_wide, in_=r_sb)
                    rw_rec = small.tile([P, CH // P], bf16, name="rw_rec", bufs=4)
                    with nc.allow_low_precision(
                        reason="softmax denominators; bf16 ok at 2e-2 gate"
                    ):
                        nc.vector.reciprocal(out=rw_rec, in_=r_wide)
                    r_row = small.tile([1, CH], bf16, name="r_row", bufs=4)
                    nc.sync.dma_start(out=r_row, in_=rw_rec)
                    pr = ps_s.tile([64, CH], f32, name="pm_s", bufs=2)
                    nc.tensor.matmul(
                        out=pr,
                        lhsT=ones_row[:, 0:64],
                        rhs=r_row,
                        start=True,
                        stop=True,
                    )
                    recipB = small.tile([64, CH], bf16, name="recipB", bufs=4)
                    nc.vector.tensor_copy(out=recipB, in_=pr)
                    nc.vector.tensor_mul(
                        out=aT[po : po + 64, t * S + c * CH : t * S + (c + 1) * CH],
                        in0=num_sb,
                        in1=recipB,
                    )

                # ---------------- the merged pipeline ----------------
                # Emission order is execution order per engine, and a tile
                # slot's new writer can only be ordered after reads that are
                # already emitted.  So: all v blocks go in iterations 0-1
                # (PV c1 needs every sk block), PV c0 chunks drain one pair
                # behind, PV c1 chunks two pairs behind — emitted BEFORE the
                # pair that reuses their ET slot (ET bufs=4) allocates.
                def drain(h, c):
                    emit_finish(h, c, emit_pv_chunk(h, c))

                for t in range(NPAIR):
                    # weight pipeline: fetch pair t+2's qk slices; w_proj
                    # spread over t=4..7
                    if t + 2 < NPAIR:
                        emit_wqk(t + 2)
                    if 4 <= t < 8:
                        emit_wp(2 * (t - 4))
                        emit_wp(2 * (t - 4) + 1)

                    # free the ET slots this pair will take over
                    if t >= 2:
                        drain(2 * (t - 2), 1)
                        drain(2 * (t - 2) + 1, 1)

                    h0, h1 = 2 * t, 2 * t + 1
                    ET[h0] = p_ET.tile([P, KT * S], bf16, name="ET")
                    ET[h1] = p_ET.tile([P, KT * S], bf16, name="ET")
                    if t > 0:
                        qk_tiles[t] = p_qk.tile([P, 2 * S], bf16, name="qk")

                    qnt, knt = nt_for_pair(t)
                    emit_b_block(qnt, qk_tiles[t], 0, cs=(1,) if t == 0 else (0, 1))
                    emit_b_block(knt, qk_tiles[t], S)
                    if t < 2:
                        for st in range(4 * t, 4 * t + 4):
                            emit_v_block(st)

                    # previous pair's first-half PV fills the PE while this
                    # pair's qk bias-casts land; breadth-first over the two
                    # heads so each head's kt4-7 scores trail their own exp
                    # chunks by a full head's worth of work
                    if t >= 1:
                        drain(2 * (t - 1), 0)
                    emit_scores_chunk(h0, range(0, 4))
                    emit_mask(h0, 0, 4)
                    emit_scores_chunk(h1, range(0, 4))
                    emit_mask(h1, 0, 4)
                    if t >= 1:
                        drain(2 * (t - 1) + 1, 0)
                    emit_scores_chunk(h0, range(4, KT))
                    emit_mask(h0, 4, KT)
                    emit_scores_chunk(h1, range(4, KT))
                    emit_mask(h1, 4, KT)

                def emit_d(st):
                    for cd in range(NCH):
                        pm = ps_b.tile([P, CH], f32, name="pm_b", bufs=2)
                        for kt in range(KT):
                            nc.tensor.matmul(
                                out=pm,
                                lhsT=aT[:, kt * S + st * P : kt * S + (st + 1) * P],
                                rhs=wp_sb[:, kt * NX + cd * CH : kt * NX + (cd + 1) * CH],
                                start=(kt == 0),
                                stop=False,
                            )
                        nc.tensor.matmul(
                            out=pm,
                            lhsT=ones_row,
                            rhs=bp_row[:, cd * CH : (cd + 1) * CH],
                            start=False,
                            stop=True,
                        )
                        stage = small.tile([P, CH], f32, name="stage", bufs=2)
                        nc.vector.tensor_copy(out=stage, in_=pm)
                        nc.sync.dma_start(
                            out=out_ext[st * P : (st + 1) * P, cd * CH : (cd + 1) * CH],
                            in_=stage,
                        )

                # tail: pair 7 c0 first (completes aT cols 0..511 for all
                # heads), then the projection's first half interleaved with
                # the remaining c1 drains.
                drain(H - 2, 0)
                drain(H - 1, 0)
                emit_d(0)
                drain(H - 4, 1)
                emit_d(1)
                drain(H - 3, 1)
                emit_d(2)
                drain(H - 2, 1)
                emit_d(3)
                drain(H - 1, 1)
                for st in range(4, ST):
                    emit_d(st)

                if dbg:
                    # only pairs 4-7's qk tiles are still live (bufs=4)
                    for t in range(4, NPAIR):
                        nc.sync.dma_start(
                            out=dbg_qkT[:, (t - 4) * 2 * S : (t - 3) * 2 * S],
                            in_=qk_tiles[t],
                        )
                    nc.sync.dma_start(out=dbg_v[:, :], in_=v_sb)
                    nc.sync.dma_start(out=dbg_ET[0, :, :], in_=ET[H - 2])
                    nc.sync.dma_start(out=dbg_ET[1, :, :], in_=ET[H - 1])
                    nc.sync.dma_start(out=dbg_aT[:, :], in_=aT)

    _split_excess_waits(nc)
    return nc


def _enable_ldw_opt():
    """walrus is invoked with --enable-ldw-opt=false on this path; turning it
    on lets codegen elide redundant LDWEIGHTS for back-to-back matmuls that
    share a stationary operand."""
    import concourse.bass_utils as bu

    if getattr(bu, "_ldw_opt_patched", False):
        return
    orig = bu.run_command

    def patched(cmd, **kw):
        cmd = [
            c.replace("--enable-ldw-opt=false", "--enable-ldw-opt=true")
            if isinstance(c, str)
            else c
            for c in cmd
        ]
        return orig(cmd, **kw)

    bu.run_command = patched
    bu._ldw_opt_patched = True


def run(inputs, trace=False, **kwargs):
    """Run the SPMD kernel on 8 cores; returns (output, BassKernelResults)."""
    from concourse.bass_utils import run_bass_kernel_spmd

    x = np.ascontiguousarray(np.asarray(inputs["x"], dtype=np.float32))
    w_attn = np.ascontiguousarray(np.asarray(inputs["w_attn"], dtype=np.float32))
    b_attn = np.ascontiguousarray(np.asarray(inputs["b_attn"], dtype=np.float32))
    w_proj = np.ascontiguousarray(np.asarray(inputs["w_proj"], dtype=np.float32))
    b_proj = np.ascontiguousarray(np.asarray(inputs["b_proj"], dtype=np.float32))

    nc = build_nc()
    in_maps = [
        {
            "x": x[b],
            "w_attn": w_attn,
            "b_attn": b_attn,
            "w_proj": w_proj,
            "b_proj": b_proj,
        }
        for b in range(B)
    ]
    res = run_bass_kernel_spmd(
        nc, in_maps, core_ids=list(range(B)), trace=trace, **kwargs
    )
    out = np.stack([res.results[i]["out"] for i in range(B)], axis=0)
    return out.astype(np.float32), res


def kernel(**inputs):
    out, _ = run(inputs)
    return out
